# revision 3
# baseline (speedup 1.0000x reference)
"""4-layer GCN (EnhancedGCN) on 8 Trainium2 NeuronCores.

Strategy (node/graph parallel):
  - Nodes sharded 12500/core across 8 cores; edges assigned to the core
    owning their dst node.
  - Each layer: h (pre-scaled by norm_src) is replicated across cores via an
    on-device AllGather; each core gathers the src rows for its edges with
    dma_gather (4 SWDGE queues, int16 bank-local indices over 4 x 25000-row
    banks), aggregates them per 128-node dst window with one-hot matmuls on
    the tensor engine (PSUM accumulation over 128-edge subtiles), applies the
    dense W matmul, then norm_dst*z + b fused on the vector engine, GELU
    (+norm_src prescale for the next layer) or the final LayerNorm, and
    writes its output shard.
  - Graph preprocessing (degree norms, edge->core/window/bank grouping,
    padding, gather index layout) happens on host once; the compiled program
    is shared by all 8 cores (SPMD), only the input data differs.
"""

import sys
import types

import numpy as np

N_NODES = 100000
N_EDGES = 1600000
D = 128
NCORES = 8
NPC = N_NODES // NCORES            # 12500 nodes per core
WINDOWS = (NPC + 127) // 128       # 98 dst windows per core (last has 84 rows)
BANKS = 4
BANK_ROWS = N_NODES // BANKS       # 25000 (int16-addressable)
import os as _os
MAX_SUB_PER_GATHER = int(_os.environ.get("KMAXSUB", "8"))  # *128 idx per gather
NQ = int(_os.environ.get("KNQ", "4"))   # SWDGE queues
SINGLE_PACKET = _os.environ.get("KSP", "0") == "1"
PAD_DLOC = 999.0

TRACE = False
LAST_EXEC_NS = None

_CACHE = {}


def _install_ntff_hook():
    if "antenv.axon_hooks" in sys.modules:
        return
    mod = types.ModuleType("antenv.axon_hooks")
    _hook = [None]
    mod.set_axon_ntff_profile_hook = lambda h: _hook.__setitem__(0, h)
    mod.get_axon_ntff_profile_hook = lambda: _hook[0]
    sys.modules["antenv.axon_hooks"] = mod
    import antenv

    antenv.axon_hooks = mod
    try:
        from trn_agent_boot.trn_boot import _ntff_profile_via_ctypes

        mod.set_axon_ntff_profile_hook(
            _ntff_profile_via_ctypes("/opt/axon/libaxon_pjrt.so")
        )
    except Exception:
        pass


def _prep_graph(src, dst):
    """Host-side graph preprocessing shared by all layers."""
    src = np.asarray(src).astype(np.int64).ravel()
    dst = np.asarray(dst).astype(np.int64).ravel()

    deg_src = np.bincount(src, minlength=N_NODES).astype(np.float64)
    deg_dst = np.bincount(dst, minlength=N_NODES).astype(np.float64)
    norm_src = np.clip(deg_src, 1.0, None) ** -0.5
    norm_dst = np.clip(deg_dst, 1.0, None) ** -0.5

    core = dst // NPC
    w = (dst % NPC) // 128
    b = src // BANK_ROWS
    group = ((core * WINDOWS + w) * BANKS + b).astype(np.int64)
    order = np.argsort(group, kind="stable")
    g_sorted = group[order]
    src_sorted = src[order]
    dst_sorted = dst[order]

    n_groups = NCORES * WINDOWS * BANKS
    counts = np.bincount(g_sorted, minlength=n_groups).reshape(
        NCORES, WINDOWS, BANKS
    )
    starts = np.zeros(n_groups + 1, np.int64)
    np.cumsum(counts.ravel(), out=starts[1:])

    # shared subtile counts: max over cores, padded to 128
    nsub_shared = np.ceil(counts.max(axis=0) / 128.0).astype(np.int64)  # [W, B]
    nsub_shared = np.maximum(nsub_shared, 1)

    # schedule (same for all cores): per window, list of gather chunks
    schedule = []  # per window: list of (bank, idxcol0, subcol0, nsub_chunk)
    idxcol = 0
    subcol = 0
    for wi in range(WINDOWS):
        gl = []
        for bi in range(BANKS):
            ns = int(nsub_shared[wi, bi])
            off = 0
            while off < ns:
                chunk = min(MAX_SUB_PER_GATHER, ns - off)
                gl.append((bi, idxcol, subcol, chunk))
                idxcol += chunk * 128 // 16
                subcol += chunk
                off += chunk
        schedule.append(gl)
    total_idxcols = idxcol
    total_subs = subcol

    per_core = []
    for c in range(NCORES):
        idx16 = np.zeros((128, total_idxcols), np.int16)
        dloc = np.full((128, total_subs), PAD_DLOC, np.float16)
        for wi in range(WINDOWS):
            by_bank = {}
            for g in schedule[wi]:
                by_bank.setdefault(g[0], []).append(g)
            for bi, chunks in by_bank.items():
                gidx = (c * WINDOWS + wi) * BANKS + bi
                s0, s1 = starts[gidx], starts[gidx + 1]
                e_src = src_sorted[s0:s1]
                e_dst = dst_sorted[s0:s1]
                n_e = s1 - s0
                cap = sum(ch[3] for ch in chunks) * 128
                assert n_e <= cap, (n_e, cap)
                loc = np.zeros(cap, np.int64)
                dl = np.full(cap, PAD_DLOC, np.float64)
                loc[:n_e] = e_src - bi * BANK_ROWS
                dl[:n_e] = (e_dst % NPC) - wi * 128
                off = 0
                for (_, icol0, scol0, chunk) in chunks:
                    nidx = chunk * 128
                    blk_loc = loc[off:off + nidx]
                    blk_dl = dl[off:off + nidx]
                    # idx layout: index i -> partition i%16, col i//16,
                    # replicated across the 8 partition stripes
                    stripe = blk_loc.reshape(nidx // 16, 16).T.astype(np.int16)
                    for s in range(8):
                        idx16[16 * s:16 * s + 16, icol0:icol0 + nidx // 16] = stripe
                    # subtile layout: edge i -> partition i%128, subtile i//128
                    dloc[:, scol0:scol0 + chunk] = (
                        blk_dl.reshape(chunk, 128).T.astype(np.float16)
                    )
                    off += nidx
        onehot = (
            dloc[:, :, None] == np.arange(128, dtype=np.float16)[None, None, :]
        )
        import ml_dtypes
        s8 = onehot.astype(ml_dtypes.float8_e4m3).reshape(128, total_subs * 128)
        per_core.append((idx16, s8))

    def node_tile(vec, c):
        full = np.zeros(WINDOWS * 128, np.float32)
        full[:NPC] = vec[c * NPC:(c + 1) * NPC].astype(np.float32)
        return full.reshape(WINDOWS, 128).T.copy()

    ns_tiles = [node_tile(norm_src, c) for c in range(NCORES)]
    ndn_tiles = [node_tile(norm_dst, c) for c in range(NCORES)]

    return schedule, total_idxcols, total_subs, per_core, ns_tiles, ndn_tiles


def _build_program(schedule, total_idxcols, total_subs):
    import os

    import concourse.bacc as bacc
    import concourse.mybir as mybir
    import concourse.tile as tile

    dbg_layers = int(os.environ.get("DBG_LAYERS", "4"))

    nc = bacc.Bacc(
        "TRN2",
        target_bir_lowering=False,
        debug=False,
        enable_asserts=False,
        num_devices=NCORES,
        num_swdge_queues=NQ,
    )
    f32, f16, i16 = mybir.dt.float32, mybir.dt.float16, mybir.dt.int16
    f8 = mybir.dt.float8e4

    x_in = nc.dram_tensor("x", [NPC, D], f32, kind="ExternalInput")
    idx_in = nc.dram_tensor("idx16", [128, total_idxcols], i16, kind="ExternalInput")
    s8_in = nc.dram_tensor("s8", [128, total_subs * D], f8, kind="ExternalInput")
    ns_in = nc.dram_tensor("ns", [128, WINDOWS], f32, kind="ExternalInput")
    ndn_in = nc.dram_tensor("ndn", [128, WINDOWS], f32, kind="ExternalInput")
    w_in = [nc.dram_tensor(f"W{i+1}", [D, D], f16, kind="ExternalInput") for i in range(4)]
    bb_in = [nc.dram_tensor(f"bb{i+1}", [128, D], f32, kind="ExternalInput") for i in range(4)]
    gam_in = nc.dram_tensor("gamma_b", [128, D], f32, kind="ExternalInput")
    bet_in = nc.dram_tensor("beta_b", [128, D], f32, kind="ExternalInput")
    out = nc.dram_tensor("out", [NPC, D], f32, kind="ExternalOutput")

    Gelu = mybir.ActivationFunctionType.Gelu
    Sqrt = mybir.ActivationFunctionType.Sqrt
    EQ = mybir.AluOpType.is_equal
    MUL = mybir.AluOpType.mult
    SUB = mybir.AluOpType.subtract
    ADD = mybir.AluOpType.add
    X = mybir.AxisListType.X

    qcnt = [0]

    with tile.TileContext(nc) as tc:
        with (
            tc.tile_pool(name="const", bufs=1) as constp,
            tc.tile_pool(name="meta", bufs=1) as metap,
            tc.tile_pool(name="xp", bufs=3) as xp,
            tc.tile_pool(name="msgp", bufs=10) as msgp,
            tc.tile_pool(name="sp", bufs=6) as sp,
            tc.tile_pool(name="aggp", bufs=4) as aggp,
            tc.tile_pool(name="hp", bufs=4) as hp,
            tc.tile_pool(name="lnp", bufs=4) as lnp,
            tc.tile_pool(name="ps1", bufs=3, space="PSUM") as ps1,
            tc.tile_pool(name="ps2", bufs=3, space="PSUM") as ps2,
            tc.tile_pool(name="dram", bufs=1, space="DRAM") as dram,
        ):
            # ---- constants / metadata into SBUF ----
            idx_sb = metap.tile([128, total_idxcols], i16)
            nc.sync.dma_start(idx_sb[:], idx_in[:])
            ns_sb = constp.tile([128, WINDOWS], f32)
            nc.sync.dma_start(ns_sb[:], ns_in[:])
            ndn_sb = constp.tile([128, WINDOWS], f32)
            nc.sync.dma_start(ndn_sb[:], ndn_in[:])
            gam_sb = constp.tile([128, D], f32)
            nc.sync.dma_start(gam_sb[:], gam_in[:])
            bet_sb = constp.tile([128, D], f32)
            nc.sync.dma_start(bet_sb[:], bet_in[:])
            w_sb = []
            bb_sb = []
            for i in range(4):
                wt = constp.tile([D, D], f16, name=f"w{i}_sb")
                nc.sync.dma_start(wt[:], w_in[i][:])
                w_sb.append(wt)
                bt = constp.tile([128, D], f32, name=f"bb{i}_sb")
                nc.sync.dma_start(bt[:], bb_in[i][:])
                bb_sb.append(bt)
            eps_t = constp.tile([128, 1], f32)
            nc.vector.memset(eps_t[:], 1e-5)

            # ---- DRAM h buffers ----
            h_shard = [
                dram.tile([NPC, D], f16, name=f"h_shard{l}") for l in range(4)
            ]
            h_full = [
                dram.tile([N_NODES, D], f16, addr_space="Shared", name=f"h_full{l}")
                for l in range(4)
            ]
            rg = [list(range(NCORES))]

            # ---- prologue: h_shard0 = x * norm_src (cast fp16) ----
            for w in range(WINDOWS):
                rows = min(128, NPC - w * 128)
                xt = xp.tile([128, D], f32, tag="xt")
                nc.sync.dma_start(xt[:rows], x_in[w * 128:w * 128 + rows, :])
                ht = xp.tile([128, D], f16, tag="ht0")
                nc.vector.tensor_scalar(
                    out=ht[:], in0=xt[:], scalar1=ns_sb[:, w:w + 1],
                    scalar2=None, op0=MUL,
                )
                nc.sync.dma_start(h_shard[0][w * 128:w * 128 + rows, :], ht[:rows])
            nc.gpsimd.collective_compute(
                "AllGather", mybir.AluOpType.bypass, replica_groups=rg,
                ins=[h_shard[0][:]], outs=[h_full[0][:]],
            )

            # ---- layers ----
            for l in range(dbg_layers):
                h_in = h_full[l]
                for w in range(WINDOWS):
                    rows = min(128, NPC - w * 128)
                    gathers = schedule[w]
                    n_tot = sum(g[3] for g in gathers)
                    psum1 = ps1.tile([128, 128], f32, tag="psum1")
                    si = 0
                    for (bi, icol0, scol0, chunk) in gathers:
                        nidx = chunk * 128
                        msg = msgp.tile([128, chunk * D], f16, tag="msg")
                        nc.gpsimd.dma_gather(
                            msg[:].rearrange("p (k d) -> p k d", d=D),
                            h_in[bi * BANK_ROWS:(bi + 1) * BANK_ROWS, :],
                            idx_sb[:, icol0:icol0 + nidx // 16],
                            nidx, nidx, D,
                            queue_num=qcnt[0] % NQ,
                            single_packet=SINGLE_PACKET,
                        )
                        qcnt[0] += 1
                        # one-hot S slab precomputed on host (fp8, 0/1)
                        s_run = sp.tile([128, chunk * D], f8, tag="s")
                        nc.sync.dma_start(
                            s_run[:],
                            s8_in[:, scol0 * D:(scol0 + chunk) * D],
                        )
                        for s in range(chunk):
                            nc.tensor.matmul(
                                psum1[:],
                                lhsT=msg[:, s * D:(s + 1) * D],
                                rhs=s_run[:, s * D:(s + 1) * D],
                                start=(si == 0), stop=(si == n_tot - 1),
                            )
                            si += 1
                    # dense: z[dst, of] = aggT.T @ W
                    aggT = aggp.tile([128, 128], f16, tag="aggT")
                    nc.scalar.copy(out=aggT[:], in_=psum1[:])
                    psum2 = ps2.tile([128, 128], f32, tag="psum2")
                    nc.tensor.matmul(psum2[:], lhsT=aggT[:], rhs=w_sb[l][:],
                                     start=True, stop=True)
                    # t2 = norm_dst * z + b  (fused on DVE)
                    t2 = hp.tile([128, D], f32, tag="t2")
                    nc.vector.scalar_tensor_tensor(
                        out=t2[:], in0=psum2[:], scalar=ndn_sb[:, w:w + 1],
                        in1=bb_sb[l][:], op0=MUL, op1=ADD,
                    )
                    if l < dbg_layers - 1:
                        g32 = hp.tile([128, D], f32, tag="g32")
                        nc.scalar.activation(out=g32[:], in_=t2[:], func=Gelu)
                        h16 = hp.tile([128, D], f16, tag="h16")
                        nc.vector.tensor_scalar(
                            out=h16[:], in0=g32[:],
                            scalar1=ns_sb[:, w:w + 1], scalar2=None, op0=MUL,
                        )
                        nc.sync.dma_start(
                            h_shard[l + 1][w * 128:w * 128 + rows, :], h16[:rows]
                        )
                    else:
                        # LayerNorm over features
                        s1 = lnp.tile([128, 1], f32, tag="s1")
                        nc.vector.reduce_sum(s1[:], t2[:], axis=X)
                        mu = lnp.tile([128, 1], f32, tag="mu")
                        nc.scalar.mul(out=mu[:], in_=s1[:], mul=1.0 / D)
                        cent = lnp.tile([128, D], f32, tag="cent")
                        nc.vector.tensor_scalar(
                            out=cent[:], in0=t2[:], scalar1=mu[:],
                            scalar2=None, op0=SUB,
                        )
                        sq = lnp.tile([128, D], f32, tag="sq")
                        nc.vector.tensor_tensor(out=sq[:], in0=cent[:],
                                                in1=cent[:], op=MUL)
                        vs = lnp.tile([128, 1], f32, tag="vs")
                        nc.vector.reduce_sum(vs[:], sq[:], axis=X)
                        std = lnp.tile([128, 1], f32, tag="std")
                        nc.scalar.activation(out=std[:], in_=vs[:], func=Sqrt,
                                             scale=1.0 / D, bias=eps_t[:])
                        rstd = lnp.tile([128, 1], f32, tag="rstd")
                        nc.vector.reciprocal(out=rstd[:], in_=std[:])
                        t1 = lnp.tile([128, D], f32, tag="t1")
                        nc.vector.tensor_scalar(out=t1[:], in0=cent[:],
                                                scalar1=rstd[:], scalar2=None,
                                                op0=MUL)
                        t4 = lnp.tile([128, D], f32, tag="t4")
                        nc.vector.tensor_tensor(out=t4[:], in0=t1[:],
                                                in1=gam_sb[:], op=MUL)
                        t5 = lnp.tile([128, D], f32, tag="t5")
                        nc.vector.tensor_tensor(out=t5[:], in0=t4[:],
                                                in1=bet_sb[:], op=ADD)
                        nc.sync.dma_start(
                            out[w * 128:w * 128 + rows, :], t5[:rows]
                        )
                if l < dbg_layers - 1:
                    nc.gpsimd.collective_compute(
                        "AllGather", mybir.AluOpType.bypass, replica_groups=rg,
                        ins=[h_shard[l + 1][:]], outs=[h_full[l + 1][:]],
                    )
    nc.compile()
    return nc


def kernel(**inputs):
    global LAST_EXEC_NS
    from concourse.bass_utils import run_bass_kernel_spmd

    x = np.asarray(inputs["x"], np.float32)
    src = inputs["src"]
    dst = inputs["dst"]

    key = "prog"
    if key not in _CACHE:
        schedule, tic, tsc, per_core, ns_tiles, ndn_tiles = _prep_graph(src, dst)
        nc = _build_program(schedule, tic, tsc)
        _CACHE[key] = (nc, per_core, ns_tiles, ndn_tiles)
    nc, per_core, ns_tiles, ndn_tiles = _CACHE[key]

    gamma = np.asarray(inputs["gamma"], np.float32).reshape(1, D)
    beta = np.asarray(inputs["beta"], np.float32).reshape(1, D)
    gamma_b = np.repeat(gamma, 128, axis=0)
    beta_b = np.repeat(beta, 128, axis=0)

    in_maps = []
    for c in range(NCORES):
        idx16, s8 = per_core[c]
        m = {
            "x": np.ascontiguousarray(x[c * NPC:(c + 1) * NPC]),
            "idx16": idx16,
            "s8": s8,
            "ns": ns_tiles[c],
            "ndn": ndn_tiles[c],
            "gamma_b": gamma_b,
            "beta_b": beta_b,
        }
        for i in range(4):
            m[f"W{i+1}"] = np.asarray(inputs[f"W{i+1}"], np.float32).astype(np.float16)
            bb = np.asarray(inputs[f"b{i+1}"], np.float32).reshape(1, D)
            m[f"bb{i+1}"] = np.repeat(bb, 128, axis=0)
        in_maps.append(m)

    if TRACE:
        _install_ntff_hook()
    res = run_bass_kernel_spmd(
        nc, in_maps, core_ids=list(range(NCORES)), trace=TRACE
    )
    LAST_EXEC_NS = res.exec_time_ns
    return np.concatenate(
        [res.results[c]["out"] for c in range(NCORES)], axis=0
    ).astype(np.float32)



# revision 26
# speedup vs baseline: 1.2058x; 1.2058x over previous
"""4-layer GCN (EnhancedGCN) on 8 Trainium2 NeuronCores.

Strategy (node/graph parallel):
  - Nodes sharded 12500/core across 8 cores; edges assigned to the core
    owning their dst node.
  - Each layer: h (pre-scaled by norm_src) is replicated across cores via an
    on-device AllGather; each core gathers the src rows for its edges with
    dma_gather (4 SWDGE queues, int16 bank-local indices over 4 x 25000-row
    banks), aggregates them per 128-node dst window with one-hot matmuls on
    the tensor engine (PSUM accumulation over 128-edge subtiles), applies the
    dense W matmul, then norm_dst*z + b fused on the vector engine, GELU
    (+norm_src prescale for the next layer) or the final LayerNorm, and
    writes its output shard.
  - Graph preprocessing (degree norms, edge->core/window/bank grouping,
    padding, gather index layout) happens on host once; the compiled program
    is shared by all 8 cores (SPMD), only the input data differs.
"""

import sys
import types

import numpy as np

N_NODES = 100000
N_EDGES = 1600000
D = 128
NCORES = 8
NPC = N_NODES // NCORES            # 12500 real nodes per core
WINDOWS = 100                      # dst windows per core (2 spare for packing)
NROWS = WINDOWS * 128              # 12800 padded rows per core (packed slots)
NFULL = NCORES * NROWS             # 102400 padded rows total
BANKS = 4
BANK_ROWS = NFULL // BANKS         # 25600 (int16-addressable)
import os as _os
MAX_SUB_PER_GATHER = int(_os.environ.get("KMAXSUB", "8"))  # *128 idx per gather
NQ = int(_os.environ.get("KNQ", "4"))   # SWDGE queues
SINGLE_PACKET = _os.environ.get("KSP", "1") == "1"
PAD_DLOC = 999.0

TRACE = False
LAST_EXEC_NS = None

_CACHE = {}


def _install_ntff_hook():
    if "antenv.axon_hooks" in sys.modules:
        return
    mod = types.ModuleType("antenv.axon_hooks")
    _hook = [None]
    mod.set_axon_ntff_profile_hook = lambda h: _hook.__setitem__(0, h)
    mod.get_axon_ntff_profile_hook = lambda: _hook[0]
    sys.modules["antenv.axon_hooks"] = mod
    import antenv

    antenv.axon_hooks = mod
    try:
        from trn_agent_boot.trn_boot import _ntff_profile_via_ctypes

        mod.set_axon_ntff_profile_hook(
            _ntff_profile_via_ctypes("/opt/axon/libaxon_pjrt.so")
        )
    except Exception:
        pass


def _pack_windows(bank_prof):
    """Greedy FFD: assign local nodes to windows, balancing per-bank in-edge
    counts toward <=512 per (window, bank) so every group packs into 4
    subtiles of 128. Returns perm (node -> packed position)."""
    npc = bank_prof.shape[0]
    cap = 512
    counts = np.zeros((WINDOWS, BANKS), np.int64)
    fill = np.zeros(WINDOWS, np.int64)          # nodes per window (<=128)
    deg = bank_prof.sum(axis=1)
    order = np.argsort(-deg, kind="stable")
    assign = np.full(npc, -1, np.int64)
    for j in order:
        p = bank_prof[j]
        newc = counts + p[None, :]
        over = np.maximum(newc - (cap - 8), 0).sum(axis=1)
        score = newc.max(axis=1) + over * 1000
        score[fill >= 128] = 1 << 40
        # LPT: place where the worst bank stays smallest
        w = int(np.lexsort((counts.sum(axis=1), score))[0])
        assign[j] = w
        counts[w] += p
        fill[w] += 1
    # repair pass: move nodes out of >cap groups into windows with room
    for _ in range(6):
        over_wb = np.argwhere(counts > cap)
        if len(over_wb) == 0:
            break
        for w, b in over_wb:
            while counts[w, b] > cap:
                members = np.where(assign == w)[0]
                members = members[bank_prof[members, b] > 0]
                if len(members) == 0:
                    break
                # move the member with the largest count in the hot bank
                j = members[np.argmax(bank_prof[members, b])]
                p = bank_prof[j]
                fits = ((counts + p[None, :] <= cap).all(axis=1)) & (fill < 128)
                fits[w] = False
                cand = np.where(fits)[0]
                if len(cand) == 0:
                    break
                # tightest destination
                w2 = cand[np.argmin(cap * BANKS - counts[cand].sum(axis=1))]
                assign[j] = w2
                counts[w] -= p
                counts[w2] += p
                fill[w] -= 1
                fill[w2] += 1
    # packed position: window-major, stable within window
    perm = np.zeros(npc, np.int64)
    for w in range(WINDOWS):
        members = np.where(assign == w)[0]
        perm[members] = w * 128 + np.arange(len(members))
    return perm, counts


def _prep_graph(src, dst):
    """Host-side graph preprocessing shared by all layers."""
    src = np.asarray(src).astype(np.int64).ravel()
    dst = np.asarray(dst).astype(np.int64).ravel()

    deg_src = np.bincount(src, minlength=N_NODES).astype(np.float64)
    deg_dst = np.bincount(dst, minlength=N_NODES).astype(np.float64)
    norm_src = np.clip(deg_src, 1.0, None) ** -0.5
    norm_dst = np.clip(deg_dst, 1.0, None) ** -0.5

    core = dst // NPC
    # per-core node->window balancing permutation (over packed positions,
    # window-major: position p -> window p//128, row p%128). A src node's
    # bank depends only on its core (BANK_ROWS == 2*NPC), never on the perm.
    perms = []
    for c in range(NCORES):
        mask = core == c
        d_loc = dst[mask] - c * NPC
        b_of_src = (src[mask] // NPC) // 2   # bank = src core pair
        prof = np.zeros((NPC, BANKS), np.int64)
        np.add.at(prof, (d_loc, b_of_src), 1)
        perm, _ = _pack_windows(prof)
        perms.append(perm)
    pos_all = np.concatenate(
        [c * NROWS + perms[c] for c in range(NCORES)]
    )  # node id -> packed flat row

    src_row = pos_all[src]                 # packed flat row of src feature
    dst_pos = pos_all[dst] % NROWS         # packed local position of dst
    w = dst_pos // 128
    b = src_row // BANK_ROWS
    group = ((core * WINDOWS + w) * BANKS + b).astype(np.int64)
    order = np.argsort(group, kind="stable")
    g_sorted = group[order]
    srcrow_sorted = src_row[order]
    dstpos_sorted = dst_pos[order]

    n_groups = NCORES * WINDOWS * BANKS
    counts = np.bincount(g_sorted, minlength=n_groups).reshape(
        NCORES, WINDOWS, BANKS
    )
    starts = np.zeros(n_groups + 1, np.int64)
    np.cumsum(counts.ravel(), out=starts[1:])

    # shared subtile counts: max over cores, padded to 128
    nsub_shared = np.ceil(counts.max(axis=0) / 128.0).astype(np.int64)  # [W, B]
    nsub_shared = np.maximum(nsub_shared, 1)

    # schedule (same for all cores): per window, list of gather chunks
    schedule = []  # per window: list of (bank, idxcol0, subcol0, nsub_chunk)
    idxcol = 0
    subcol = 0
    for wi in range(WINDOWS):
        gl = []
        for bi in range(BANKS):
            ns = int(nsub_shared[wi, bi])
            off = 0
            while off < ns:
                chunk = min(MAX_SUB_PER_GATHER, ns - off)
                gl.append((bi, idxcol, subcol, chunk))
                idxcol += chunk * 128 // 16
                subcol += chunk
                off += chunk
        schedule.append(gl)
    total_idxcols = idxcol
    total_subs = subcol

    per_core = []
    for c in range(NCORES):
        idx16 = np.zeros((128, total_idxcols), np.int16)
        dloc = np.full((128, total_subs), PAD_DLOC, np.float16)
        for wi in range(WINDOWS):
            by_bank = {}
            for g in schedule[wi]:
                by_bank.setdefault(g[0], []).append(g)
            for bi, chunks in by_bank.items():
                gidx = (c * WINDOWS + wi) * BANKS + bi
                s0, s1 = starts[gidx], starts[gidx + 1]
                e_srcrow = srcrow_sorted[s0:s1]
                e_dstpos = dstpos_sorted[s0:s1]
                n_e = s1 - s0
                cap = sum(ch[3] for ch in chunks) * 128
                assert n_e <= cap, (n_e, cap)
                loc = np.zeros(cap, np.int64)
                dl = np.full(cap, PAD_DLOC, np.float64)
                loc[:n_e] = e_srcrow - bi * BANK_ROWS
                dl[:n_e] = e_dstpos - wi * 128
                off = 0
                for (_, icol0, scol0, chunk) in chunks:
                    nidx = chunk * 128
                    blk_loc = loc[off:off + nidx]
                    blk_dl = dl[off:off + nidx]
                    # idx layout: index i -> partition i%16, col i//16,
                    # replicated across the 8 partition stripes
                    stripe = blk_loc.reshape(nidx // 16, 16).T.astype(np.int16)
                    for s in range(8):
                        idx16[16 * s:16 * s + 16, icol0:icol0 + nidx // 16] = stripe
                    # subtile layout: edge i -> partition i%128, subtile i//128
                    dloc[:, scol0:scol0 + chunk] = (
                        blk_dl.reshape(chunk, 128).T.astype(np.float16)
                    )
                    off += nidx
        onehot = (
            dloc[:, :, None] == np.arange(128, dtype=np.float16)[None, None, :]
        )
        import ml_dtypes
        s8 = onehot.astype(ml_dtypes.float8_e4m3).reshape(128, total_subs * 128)
        per_core.append((idx16, s8))

    def node_tile(vec, c):
        # packed position perms[c][j] holds local node j; pad slots -> 0
        full = np.zeros(NROWS, np.float32)
        full[perms[c]] = vec[c * NPC:(c + 1) * NPC].astype(np.float32)
        return full.reshape(WINDOWS, 128).T.copy()

    ns_tiles = [node_tile(norm_src, c) for c in range(NCORES)]
    ndn_tiles = [node_tile(norm_dst, c) for c in range(NCORES)]

    return (schedule, total_idxcols, total_subs, per_core, ns_tiles, ndn_tiles,
            perms)


def _build_program(schedule, total_idxcols, total_subs):
    import os

    import concourse.bacc as bacc
    import concourse.mybir as mybir
    import concourse.tile as tile

    dbg_layers = int(os.environ.get("DBG_LAYERS", "4"))

    nc = bacc.Bacc(
        "TRN2",
        target_bir_lowering=False,
        debug=False,
        enable_asserts=False,
        num_devices=NCORES,
        num_swdge_queues=NQ,
    )
    f32, f16, i16 = mybir.dt.float32, mybir.dt.float16, mybir.dt.int16
    f8 = mybir.dt.float8e4

    x_in = nc.dram_tensor("x", [NROWS, D], f32, kind="ExternalInput")
    idx_in = nc.dram_tensor("idx16", [128, total_idxcols], i16, kind="ExternalInput")
    s8_in = nc.dram_tensor("s8", [128, total_subs * D], f8, kind="ExternalInput")
    ns_in = nc.dram_tensor("ns", [128, WINDOWS], f32, kind="ExternalInput")
    ndn_in = nc.dram_tensor("ndn", [128, WINDOWS], f32, kind="ExternalInput")
    w_in = [nc.dram_tensor(f"W{i+1}", [D, D], f16, kind="ExternalInput") for i in range(4)]
    bb_in = [nc.dram_tensor(f"bb{i+1}", [128, D], f32, kind="ExternalInput") for i in range(4)]
    gam_in = nc.dram_tensor("gamma_b", [128, D], f32, kind="ExternalInput")
    bet_in = nc.dram_tensor("beta_b", [128, D], f32, kind="ExternalInput")
    out = nc.dram_tensor("out", [NROWS, D], f32, kind="ExternalOutput")

    Gelu = mybir.ActivationFunctionType.Gelu
    Sqrt = mybir.ActivationFunctionType.Sqrt
    EQ = mybir.AluOpType.is_equal
    MUL = mybir.AluOpType.mult
    SUB = mybir.AluOpType.subtract
    ADD = mybir.AluOpType.add
    X = mybir.AxisListType.X

    qcnt = [0]

    with tile.TileContext(nc) as tc:
        with (
            tc.tile_pool(name="const", bufs=1) as constp,
            tc.tile_pool(name="meta", bufs=1) as metap,
            tc.tile_pool(name="xp", bufs=3) as xp,
            tc.tile_pool(name="msgp", bufs=10) as msgp,
            tc.tile_pool(name="sp", bufs=6) as sp,
            tc.tile_pool(name="aggp", bufs=4) as aggp,
            tc.tile_pool(name="hp", bufs=4) as hp,
            tc.tile_pool(name="lnp", bufs=4) as lnp,
            tc.tile_pool(name="ps1", bufs=3, space="PSUM") as ps1,
            tc.tile_pool(name="ps2", bufs=3, space="PSUM") as ps2,
            tc.tile_pool(name="dram", bufs=1, space="DRAM") as dram,
        ):
            # ---- constants / metadata into SBUF ----
            idx_sb = metap.tile([128, total_idxcols], i16)
            nc.sync.dma_start(idx_sb[:], idx_in[:])
            ns_sb = constp.tile([128, WINDOWS], f32)
            nc.sync.dma_start(ns_sb[:], ns_in[:])
            ndn_sb = constp.tile([128, WINDOWS], f32)
            nc.sync.dma_start(ndn_sb[:], ndn_in[:])
            gam_sb = constp.tile([128, D], f32)
            nc.sync.dma_start(gam_sb[:], gam_in[:])
            bet_sb = constp.tile([128, D], f32)
            nc.sync.dma_start(bet_sb[:], bet_in[:])
            w_sb = []
            bb_sb = []
            for i in range(4):
                wt = constp.tile([D, D], f16, name=f"w{i}_sb")
                nc.sync.dma_start(wt[:], w_in[i][:])
                w_sb.append(wt)
                bt = constp.tile([128, D], f32, name=f"bb{i}_sb")
                nc.sync.dma_start(bt[:], bb_in[i][:])
                bb_sb.append(bt)
            eps_t = constp.tile([128, 1], f32)
            nc.vector.memset(eps_t[:], 1e-5)

            # ---- DRAM h buffers ----
            h_shard = [
                dram.tile([NROWS, D], f16, name=f"h_shard{l}") for l in range(4)
            ]
            h_full = [
                dram.tile([NFULL, D], f16, addr_space="Shared", name=f"h_full{l}")
                for l in range(4)
            ]
            rg = [list(range(NCORES))]

            # ---- prologue: h_shard0 = x * norm_src (cast fp16) ----
            for w in range(WINDOWS):
                xt = xp.tile([128, D], f32, tag="xt")
                nc.sync.dma_start(xt[:], x_in[w * 128:(w + 1) * 128, :])
                ht = xp.tile([128, D], f16, tag="ht0")
                nc.vector.tensor_scalar(
                    out=ht[:], in0=xt[:], scalar1=ns_sb[:, w:w + 1],
                    scalar2=None, op0=MUL,
                )
                nc.sync.dma_start(h_shard[0][w * 128:(w + 1) * 128, :], ht[:])
            nc.gpsimd.collective_compute(
                "AllGather", mybir.AluOpType.bypass, replica_groups=rg,
                ins=[h_shard[0][:]], outs=[h_full[0][:]],
            )

            # ---- layers ----
            for l in range(dbg_layers):
                h_in = h_full[l]
                for w in range(WINDOWS):
                    rows = 128
                    gathers = schedule[w]
                    n_tot = sum(g[3] for g in gathers)
                    psum1 = ps1.tile([128, 128], f32, tag="psum1")
                    si = 0
                    for (bi, icol0, scol0, chunk) in gathers:
                        nidx = chunk * 128
                        msg = msgp.tile([128, chunk * D], f16, tag="msg")
                        nc.gpsimd.dma_gather(
                            msg[:].rearrange("p (k d) -> p k d", d=D),
                            h_in[bi * BANK_ROWS:(bi + 1) * BANK_ROWS, :],
                            idx_sb[:, icol0:icol0 + nidx // 16],
                            nidx, nidx, D,
                            queue_num=qcnt[0] % NQ,
                            single_packet=SINGLE_PACKET,
                        )
                        qcnt[0] += 1
                        # one-hot S slab precomputed on host (fp8, 0/1)
                        s_run = sp.tile([128, chunk * D], f8, tag="s")
                        nc.sync.dma_start(
                            s_run[:],
                            s8_in[:, scol0 * D:(scol0 + chunk) * D],
                        )
                        for s in range(chunk):
                            nc.tensor.matmul(
                                psum1[:],
                                lhsT=msg[:, s * D:(s + 1) * D],
                                rhs=s_run[:, s * D:(s + 1) * D],
                                start=(si == 0), stop=(si == n_tot - 1),
                            )
                            si += 1
                    # dense: z[dst, of] = aggT.T @ W
                    aggT = aggp.tile([128, 128], f16, tag="aggT")
                    nc.scalar.copy(out=aggT[:], in_=psum1[:])
                    psum2 = ps2.tile([128, 128], f32, tag="psum2")
                    nc.tensor.matmul(psum2[:], lhsT=aggT[:], rhs=w_sb[l][:],
                                     start=True, stop=True)
                    # t2 = norm_dst * z + b  (fused on DVE)
                    t2 = hp.tile([128, D], f32, tag="t2")
                    nc.vector.scalar_tensor_tensor(
                        out=t2[:], in0=psum2[:], scalar=ndn_sb[:, w:w + 1],
                        in1=bb_sb[l][:], op0=MUL, op1=ADD,
                    )
                    if l < dbg_layers - 1:
                        g32 = hp.tile([128, D], f32, tag="g32")
                        nc.scalar.activation(out=g32[:], in_=t2[:], func=Gelu)
                        h16 = hp.tile([128, D], f16, tag="h16")
                        nc.vector.tensor_scalar(
                            out=h16[:], in0=g32[:],
                            scalar1=ns_sb[:, w:w + 1], scalar2=None, op0=MUL,
                        )
                        nc.sync.dma_start(
                            h_shard[l + 1][w * 128:w * 128 + rows, :], h16[:rows]
                        )
                    else:
                        # LayerNorm over features
                        s1 = lnp.tile([128, 1], f32, tag="s1")
                        nc.vector.reduce_sum(s1[:], t2[:], axis=X)
                        mu = lnp.tile([128, 1], f32, tag="mu")
                        nc.scalar.mul(out=mu[:], in_=s1[:], mul=1.0 / D)
                        cent = lnp.tile([128, D], f32, tag="cent")
                        nc.vector.tensor_scalar(
                            out=cent[:], in0=t2[:], scalar1=mu[:],
                            scalar2=None, op0=SUB,
                        )
                        sq = lnp.tile([128, D], f32, tag="sq")
                        nc.vector.tensor_tensor(out=sq[:], in0=cent[:],
                                                in1=cent[:], op=MUL)
                        vs = lnp.tile([128, 1], f32, tag="vs")
                        nc.vector.reduce_sum(vs[:], sq[:], axis=X)
                        std = lnp.tile([128, 1], f32, tag="std")
                        nc.scalar.activation(out=std[:], in_=vs[:], func=Sqrt,
                                             scale=1.0 / D, bias=eps_t[:])
                        rstd = lnp.tile([128, 1], f32, tag="rstd")
                        nc.vector.reciprocal(out=rstd[:], in_=std[:])
                        t1 = lnp.tile([128, D], f32, tag="t1")
                        nc.vector.tensor_scalar(out=t1[:], in0=cent[:],
                                                scalar1=rstd[:], scalar2=None,
                                                op0=MUL)
                        t4 = lnp.tile([128, D], f32, tag="t4")
                        nc.vector.tensor_tensor(out=t4[:], in0=t1[:],
                                                in1=gam_sb[:], op=MUL)
                        t5 = lnp.tile([128, D], f32, tag="t5")
                        nc.vector.tensor_tensor(out=t5[:], in0=t4[:],
                                                in1=bet_sb[:], op=ADD)
                        nc.sync.dma_start(
                            out[w * 128:w * 128 + rows, :], t5[:rows]
                        )
                if l < dbg_layers - 1:
                    nc.gpsimd.collective_compute(
                        "AllGather", mybir.AluOpType.bypass, replica_groups=rg,
                        ins=[h_shard[l + 1][:]], outs=[h_full[l + 1][:]],
                    )
    nc.compile()
    return nc


def kernel(**inputs):
    global LAST_EXEC_NS
    from concourse.bass_utils import run_bass_kernel_spmd

    x = np.asarray(inputs["x"], np.float32)
    src = inputs["src"]
    dst = inputs["dst"]

    key = "prog"
    if key not in _CACHE:
        (schedule, tic, tsc, per_core, ns_tiles, ndn_tiles,
         perms) = _prep_graph(src, dst)
        nc = _build_program(schedule, tic, tsc)
        _CACHE[key] = (nc, per_core, ns_tiles, ndn_tiles, perms)
    nc, per_core, ns_tiles, ndn_tiles, perms = _CACHE[key]

    gamma = np.asarray(inputs["gamma"], np.float32).reshape(1, D)
    beta = np.asarray(inputs["beta"], np.float32).reshape(1, D)
    gamma_b = np.repeat(gamma, 128, axis=0)
    beta_b = np.repeat(beta, 128, axis=0)

    in_maps = []
    for c in range(NCORES):
        idx16, s8 = per_core[c]
        x_packed = np.zeros((NROWS, D), np.float32)
        x_packed[perms[c]] = x[c * NPC:(c + 1) * NPC]
        m = {
            "x": x_packed,
            "idx16": idx16,
            "s8": s8,
            "ns": ns_tiles[c],
            "ndn": ndn_tiles[c],
            "gamma_b": gamma_b,
            "beta_b": beta_b,
        }
        for i in range(4):
            m[f"W{i+1}"] = np.asarray(inputs[f"W{i+1}"], np.float32).astype(np.float16)
            bb = np.asarray(inputs[f"b{i+1}"], np.float32).reshape(1, D)
            m[f"bb{i+1}"] = np.repeat(bb, 128, axis=0)
        in_maps.append(m)

    if TRACE:
        _install_ntff_hook()
    res = run_bass_kernel_spmd(
        nc, in_maps, core_ids=list(range(NCORES)), trace=TRACE
    )
    LAST_EXEC_NS = res.exec_time_ns
    return np.concatenate(
        [res.results[c]["out"][perms[c]] for c in range(NCORES)], axis=0
    ).astype(np.float32)

